# revision 3
# baseline (speedup 1.0000x reference)
"""MoE MLP kernel for Trainium2 (8 NeuronCores, Bass/Tile) — v2.

Problem: y = concat(h @ W2 + b2, h @ We[idx_b] + be[idx_b]) where
h = gelu(x @ W1 + b1), x: [16, 2048, 1024] f32, W1: [1024, 4096],
W2: [4096, 768], We: [8, 4096, 256], idx: [16] in [0, 8).

Sharding: data-parallel over batch B=16 -> 2 batch elements per core.
Expert selection resolved on host (indices are host-visible): each core
gets [W2 | We_sel] pre-concatenated per batch element -> one dense GEMM
pipeline, perfectly balanced, no collectives.

v2 changes vs v1:
  * all matmul operands in bf16 (same 1 cycle/row PE rate as fp32r, but
    half the DMA bytes and half the SBUF footprint; PSUM accum is f32)
  * the fused [W2 | We_sel] weight matrix stays RESIDENT in SBUF per
    batch element (64 KiB/partition in bf16) instead of being
    re-streamed from DRAM for every token tile and column pass: wcat
    DMA drops 134 MB -> 16.8 MB per exec per core.
  * runner uses fast_dispatch_compile (C++ no-effect dispatch path)
    when available: the default effectful path pays a multi-ms python
    dispatch tax per call through the axon tunnel.

Per-core pipeline (4096 tokens, token tiles of TT=1024):
  Phase A: hT[hid, tok] = gelu(W1.T @ xT + b1); x pre-transposed on
    host so the contraction dim (IN) lies on SBUF partitions for both
    operands; hT comes out HID-major = the lhsT layout phase B needs.
    8 K-chunks accumulate in PSUM; ScalarE applies bias+gelu (erf-exact)
    on the PSUM->SBUF move, writing bf16.
  Phase B: out[tok, col] = hT.T @ Wcat + bcat, Wcat resident in SBUF.
    Token halves (u) outer: 4 token-groups x 2 col-halves fill the 8
    PSUM banks; each stationary hT chunk feeds both column halves
    back-to-back (stationary reload amortized). VectorE fuses bias-add
    with the PSUM->SBUF copy; outputs land token-major -> straight DMA
    to DRAM.
"""

import sys

sys.path.insert(0, "/opt/trn_rl_repo")

import numpy as np
import ml_dtypes

import concourse.bass as bass  # noqa: F401  (engine namespaces live on nc)
import concourse.mybir as mybir
import concourse.tile as tile
from concourse import bacc, bass2jax

# Problem dims (hardcoded per contract)
IN, HID, OUT, PART, E = 1024, 4096, 1024, 256, 8
B, N_TOK = 16, 2048
NCORES = 8
BPC = B // NCORES            # batch elements per core
T_CORE = BPC * N_TOK         # tokens per core
TT = 1024                    # token tile
NT = T_CORE // TT            # token tiles per core
TPB = N_TOK // TT            # token tiles per batch element
KC = IN // 128               # fc1 contraction chunks
MC = HID // 128              # hidden chunks
F32 = mybir.dt.float32
BF16 = mybir.dt.bfloat16
NP_BF16 = ml_dtypes.bfloat16

_CACHE = {}


def _strip_redundant_ldweights(nc):
    """Remove InstLdweights whose stationary AP is identical to the
    immediately-preceding Ldweights on the PE stream with only plain
    matmuls in between: the PE array already holds those weights, so the
    reload is pure overhead (~128 cols of SBUF streaming per instance)
    that the cost model doesn't even charge for. Dependencies of a
    stripped load are merged into the following matmul."""
    def ldkey(inst):
        a = inst.ins[0]
        return (
            str(a.memref), a.offset, str(a.ap), str(a.dtype),
            str(inst.perf_mode), str(inst.is_transpose),
        )

    n_stripped = 0
    for blk in nc.m.functions[0].blocks:
        il = blk.instructions  # live list
        prev_key = None
        only_mm_since = True
        i = 0
        while i < len(il):
            inst = il[i]
            if getattr(inst, "engine", None) != mybir.EngineType.PE:
                i += 1
                continue
            if isinstance(inst, mybir.InstLdweights):
                k = ldkey(inst)
                nxt = il[i + 1] if i + 1 < len(il) else None
                if (
                    k == prev_key
                    and only_mm_since
                    and not inst.has_wait()
                    and not inst.has_update()
                    and isinstance(nxt, mybir.InstMatmult)
                ):
                    try:
                        nxt.merge_dependencies_from(inst)
                    except Exception:
                        nxt.add_sync_dependencies_from(inst)
                        nxt.add_nosync_dependencies_from(inst)
                    del il[i]
                    n_stripped += 1
                    continue  # don't advance; don't reset prev_key
                prev_key = k
                only_mm_since = True
            elif isinstance(inst, mybir.InstMatmult):
                pass
            else:
                only_mm_since = False
            i += 1
    return n_stripped


STRIP_LDW = True


def _build_nc(reps=1, hw_loop=False):
    """reps>1 repeats the full computation in one NEFF (timing variant:
    device time scales with reps while per-call dispatch cost does not).
    hw_loop=True wraps the repetition in a For_i hardware loop instead of
    unrolling (constant NEFF size for any reps)."""
    nc = bacc.Bacc(None, target_bir_lowering=False, debug=False)

    xt_d = nc.dram_tensor("xt", [IN, T_CORE], BF16, kind="ExternalInput")
    w1_d = nc.dram_tensor("w1", [MC, 128, IN], BF16, kind="ExternalInput")
    b1_d = nc.dram_tensor("b1r", [128, MC], F32, kind="ExternalInput")
    wb_d = nc.dram_tensor("wb", [BPC, MC, 128, OUT], BF16, kind="ExternalInput")
    bb_d = nc.dram_tensor("bb", [BPC, 128, OUT], F32, kind="ExternalInput")
    out_d = nc.dram_tensor("out", [T_CORE, OUT], F32, kind="ExternalOutput")

    with tile.TileContext(nc) as tc:
        with (
            tc.tile_pool(name="const", bufs=1) as cpool,
            tc.tile_pool(name="h", bufs=MC) as hpool,
            tc.tile_pool(name="x", bufs=2 * KC) as xpool,
            tc.tile_pool(name="w1", bufs=4) as w1pool,
            tc.tile_pool(name="wb", bufs=MC) as wbpool,
            tc.tile_pool(name="o", bufs=4) as opool,
            tc.tile_pool(name="ps", bufs=8, space="PSUM") as pspool,
        ):
            b1_sb = cpool.tile([128, MC], F32, tag="b1")
            nc.sync.dma_start(b1_sb[:], b1_d[:])
            bb_sb = []
            for j in range(BPC):
                t_ = cpool.tile([128, OUT], F32, tag=f"bb{j}")
                nc.sync.dma_start(t_[:], bb_d[j])
                bb_sb.append(t_)

            def one_rep():
                for j in range(BPC):
                    # resident [W2 | We_sel] for this batch element:
                    # 32 x [128, 1024] bf16 = 64 KiB/partition. Loads are
                    # interleaved into the first tile's m-loop below so the
                    # 8.4 MB blob doesn't delay the critical x/w1 DMAs.
                    wcat = [None] * MC

                    for tt_i in range(TPB):
                        t = j * TPB + tt_i

                        # ---- Phase A: hT = gelu(W1.T @ xT + b1) ----
                        xts = []
                        for k in range(KC):
                            xk = xpool.tile([128, TT], BF16, tag="x")
                            nc.sync.dma_start(
                                xk[:],
                                xt_d[k * 128 : (k + 1) * 128, t * TT : (t + 1) * TT],
                            )
                            xts.append(xk)

                        hts = []
                        for m in range(MC):
                            w1m = w1pool.tile([128, IN], BF16, tag="w1")
                            nc.sync.dma_start(w1m[:], w1_d[m])
                            if tt_i == 0:
                                wt = wbpool.tile([128, OUT], BF16, tag="wb")
                                nc.sync.dma_start(wt[:], wb_d[j, m])
                                wcat[m] = wt
                            hm = hpool.tile([128, TT], BF16, tag="h")
                            # k-outer / s-inner: each stationary W1 chunk feeds
                            # the two 512-token subtiles back-to-back
                            psa = [
                                pspool.tile([128, 512], F32, tag="ps", name=f"psa{s}")
                                for s in range(TT // 512)
                            ]
                            for k in range(KC):
                                for s in range(TT // 512):
                                    nc.tensor.matmul(
                                        psa[s][:],
                                        w1m[:, k * 128 : (k + 1) * 128],
                                        xts[k][:, s * 512 : (s + 1) * 512],
                                        start=(k == 0),
                                        stop=(k == KC - 1),
                                    )
                            for s in range(TT // 512):
                                nc.scalar.activation(
                                    hm[:, s * 512 : (s + 1) * 512],
                                    psa[s][:],
                                    mybir.ActivationFunctionType.Gelu,
                                    bias=b1_sb[:, m : m + 1],
                                )
                            hts.append(hm)

                        # ---- Phase B: out = hT.T @ Wcat + bias ----
                        # Token halves (u) outer; per m the resident Wcat slab
                        # is the moving operand and each stationary hT chunk
                        # (m, g) feeds both column halves back-to-back.
                        # 4 token-groups x 2 col-halves = 8 PSUM banks.
                        # g-outer: each token-group's 2 banks finish their
                        # m-chain a quarter-pass early, so the bias-add +
                        # store drain overlaps the remaining matmuls (shrinks
                        # the end-of-kernel drain tail).
                        for u in range(2):
                            pso = [
                                pspool.tile([128, 512], F32, tag="ps", name=f"pso{i}")
                                for i in range(8)
                            ]
                            for g in range(4):
                                for m in range(MC):
                                    for h in range(2):
                                        nc.tensor.matmul(
                                            pso[g * 2 + h][:],
                                            hts[m][
                                                :,
                                                u * 512
                                                + g * 128 : u * 512
                                                + (g + 1) * 128,
                                            ],
                                            wcat[m][:, h * 512 : (h + 1) * 512],
                                            start=(m == 0),
                                            stop=(m == MC - 1),
                                        )
                                ob = opool.tile([128, OUT], F32, tag="o")
                                for h in range(2):
                                    nc.vector.tensor_add(
                                        ob[:, h * 512 : (h + 1) * 512],
                                        pso[g * 2 + h][:],
                                        bb_sb[j][:, h * 512 : (h + 1) * 512],
                                    )
                                row0 = t * TT + u * 512 + g * 128
                                nc.sync.dma_start(
                                    out_d[row0 : row0 + 128, :], ob[:]
                                )

            if hw_loop:
                with tc.For_i(0, reps):
                    one_rep()
            else:
                for _ in range(reps):
                    one_rep()

    if STRIP_LDW:
        n = _strip_redundant_ldweights(nc)
        print(f"stripped {n} redundant ldweights")
    nc.compile()
    return nc


def _make_runner(nc):
    """Cached executor mirroring bass2jax.run_bass_via_pjrt's multi-core
    path, but reusable: the jitted body + device-resident inputs persist
    across calls so repeat executions measure device time, not transfers.
    Uses the C++ fast dispatch path (no bass_effect) when available."""
    import jax
    from jax.experimental.shard_map import shard_map
    from jax.sharding import Mesh, NamedSharding, PartitionSpec

    bass2jax.install_neuronx_cc_hook()

    partition_name = (
        nc.partition_id_tensor.name if nc.partition_id_tensor else None
    )
    in_names, out_names, out_avals, zero_outs = [], [], [], []
    for alloc in nc.m.functions[0].allocations:
        if not isinstance(alloc, mybir.MemoryLocationSet):
            continue
        name = alloc.memorylocations[0].name
        if alloc.kind == "ExternalInput":
            if name != partition_name:
                in_names.append(name)
        elif alloc.kind == "ExternalOutput":
            out_avals.append(
                jax.core.ShapedArray(alloc.tensor_shape, mybir.dt.np(alloc.dtype))
            )
            zero_outs.append(
                np.zeros(alloc.tensor_shape, dtype=mybir.dt.np(alloc.dtype))
            )
            out_names.append(name)

    n_params = len(in_names)
    all_names = in_names + out_names
    if partition_name is not None:
        all_names = all_names + [partition_name]

    def _body(*args):
        operands = list(args)
        if partition_name is not None:
            operands.append(bass2jax.partition_id_tensor())
        outs = bass2jax._bass_exec_p.bind(
            *operands,
            out_avals=tuple(out_avals),
            in_names=tuple(all_names),
            out_names=tuple(out_names),
            lowering_input_output_aliases=(),
            sim_require_finite=True,
            sim_require_nnan=True,
            nc=nc,
        )
        return tuple(outs)

    devices = jax.devices()[:NCORES]
    mesh = Mesh(np.asarray(devices), ("core",))
    spec = NamedSharding(mesh, PartitionSpec("core"))
    n_outs = len(out_names)

    def make_jit():
        return jax.jit(
            shard_map(
                _body,
                mesh=mesh,
                in_specs=(PartitionSpec("core"),) * (n_params + n_outs),
                out_specs=(PartitionSpec("core"),) * n_outs,
                check_rep=False,
            ),
            donate_argnums=tuple(range(n_params, n_params + n_outs)),
            keep_unused=True,
        )

    def _global_struct(per_core_shape, dtype):
        return jax.ShapeDtypeStruct(
            (NCORES * per_core_shape[0], *per_core_shape[1:]), dtype, sharding=spec
        )

    in_structs = None  # filled lazily from the first put_inputs call

    def put_inputs(in_maps):
        concat = [
            np.concatenate([np.asarray(m[name]) for m in in_maps], axis=0)
            for name in in_names
        ]
        return [jax.device_put(a, spec) for a in concat]

    def put_zeros():
        return [
            jax.device_put(
                np.zeros((NCORES * z.shape[0], *z.shape[1:]), z.dtype), spec
            )
            for z in zero_outs
        ]

    # compile once, preferring the fast (effect-free) dispatch path
    sharded = None
    try:
        # build arg structs from the BIR-declared shapes (inputs then outs)
        arg_structs = []
        for name in in_names:
            for alloc in nc.m.functions[0].allocations:
                if (
                    isinstance(alloc, mybir.MemoryLocationSet)
                    and alloc.memorylocations[0].name == name
                ):
                    arg_structs.append(
                        _global_struct(alloc.tensor_shape, mybir.dt.np(alloc.dtype))
                    )
                    break
        for z in zero_outs:
            arg_structs.append(_global_struct(z.shape, z.dtype))
        sharded = bass2jax.fast_dispatch_compile(
            lambda: make_jit().lower(*arg_structs).compile()
        )
    except Exception as e:  # pragma: no cover - fallback for older repo revs
        print(f"fast_dispatch unavailable ({type(e).__name__}: {e}); using jit")
        sharded = make_jit()

    def run(dev_inputs, dev_zeros):
        out_arrs = sharded(*dev_inputs, *dev_zeros)
        return [
            {
                name: np.asarray(out_arrs[i]).reshape(
                    NCORES, *out_avals[i].shape
                )[c]
                for i, name in enumerate(out_names)
            }
            for c in range(NCORES)
        ]

    return {
        "run": run,
        "put_inputs": put_inputs,
        "put_zeros": put_zeros,
        "sharded": sharded,
        "out_names": out_names,
    }


def get_runner():
    if "nc" not in _CACHE:
        _CACHE["nc"] = _build_nc()
    if "runner" not in _CACHE:
        _CACHE["runner"] = _make_runner(_CACHE["nc"])
    return _CACHE["runner"]


def make_in_maps(x, indices, W1, b1, W2, b2, We, be):
    # Replicated weights, rearranged so every DMA is a contiguous slab:
    # w1r[m, p, k*128+q] = W1[k*128+p, m*128+q]
    w1r = np.ascontiguousarray(
        W1.reshape(KC, 128, MC, 128).transpose(2, 1, 0, 3).reshape(MC, 128, IN)
    ).astype(NP_BF16)
    b1r = np.ascontiguousarray(b1.reshape(MC, 128).T)

    in_maps = []
    for c in range(NCORES):
        xt = np.ascontiguousarray(
            x[c * BPC : (c + 1) * BPC].reshape(T_CORE, IN).T
        ).astype(NP_BF16)
        wb = np.empty((BPC, MC, 128, OUT), dtype=NP_BF16)
        bb = np.empty((BPC, 128, OUT), dtype=np.float32)
        for jj in range(BPC):
            e = int(indices[c * BPC + jj])
            wcat = np.concatenate([W2, We[e]], axis=1)  # [HID, OUT]
            wb[jj] = wcat.reshape(MC, 128, OUT).astype(NP_BF16)
            bb[jj] = np.concatenate([b2, be[e]])[None, :]
        in_maps.append({"xt": xt, "w1": w1r, "b1r": b1r, "wb": wb, "bb": bb})
    return in_maps


def kernel(x, indices, W1, b1, W2, b2, We, be):
    x = np.ascontiguousarray(np.asarray(x, dtype=np.float32))
    indices = np.asarray(indices).astype(np.int64)
    W1 = np.asarray(W1, dtype=np.float32)
    b1 = np.asarray(b1, dtype=np.float32)
    W2 = np.asarray(W2, dtype=np.float32)
    b2 = np.asarray(b2, dtype=np.float32)
    We = np.asarray(We, dtype=np.float32)
    be = np.asarray(be, dtype=np.float32)

    runner = get_runner()
    in_maps = make_in_maps(x, indices, W1, b1, W2, b2, We, be)
    dev_in = runner["put_inputs"](in_maps)
    results = runner["run"](dev_in, runner["put_zeros"]())

    out = np.empty((B, N_TOK, OUT), dtype=np.float32)
    for c in range(NCORES):
        out[c * BPC : (c + 1) * BPC] = results[c]["out"].reshape(BPC, N_TOK, OUT)
    return out



# revision 4
# speedup vs baseline: 1.1881x; 1.1881x over previous
"""MoE MLP kernel for Trainium2 (8 NeuronCores, Bass/Tile) — v3.

Problem: y = concat(h @ W2 + b2, h @ We[idx_b] + be[idx_b]) where
h = gelu(x @ W1 + b1), x: [16, 2048, 1024] f32, W1: [1024, 4096],
W2: [4096, 768], We: [8, 4096, 256], idx: [16] in [0, 8).

Sharding: data-parallel over batch B=16 -> 2 batch elements per core.
Expert selection resolved on host: each core gets [W2 | We_sel] packed
per batch element -> one dense GEMM pipeline, no collectives.

v3 changes vs v2:
  * token tile TT=2048 = one full batch element per tile, so every
    stationary weight chunk feeds FOUR N=512 matmuls (was two) in both
    phases -> half the LDWEIGHTS instructions.
  * redundant LDWEIGHTS (same stationary AP, only matmuls in between)
    are stripped post-schedule: LDWEIGHTS is unmodeled in the cost
    model but costs ~107ns of real PE time each; 4096 -> 1024 remain.
  * phase B swaps stationary/moving: stationary = Wcat column-chunk
    [128 hid, 128 col] (prepacked per (batch, col-chunk) as one
    contiguous [128, 4096] slab = one 1 MB DMA), moving = h token
    slices. Output lands column-major [OUT, N_TOK]; host untransposes.
  * outputs stored bf16 (halves output DMA; rounding adds ~0.2% per
    element, far inside the 2e-2 gate).
  * per-phase DMA per core drops to x 8.4 + W1 16.8 + Wcat 16.8 +
    out 8.4 = 50 MB/rep, fully hidden under ~0.9 ms of PE work.

Per-core pipeline per batch element (2048 tokens):
  Phase A: for m (32 hid chunks): psum[s], s=0..3 accumulate over k=8
    IN-chunks; each stationary W1 (m,k) chunk feeds 4 token subtiles
    back-to-back. ScalarE applies bias+gelu on the PSUM->SBUF move,
    writing h[m] [128, 2048] bf16 (128 KiB/partition total, resident).
  Phase B: for c (8 col chunks): psum[u], u=0..3 accumulate over m=32
    hid chunks; each stationary Wcat (c,m) chunk feeds 4 token subtiles.
    ScalarE fuses bias-add with the PSUM->SBUF copy (bf16); one 512 KB
    DMA per (batch, col chunk) stores [128, 2048] to DRAM.
"""

import sys

sys.path.insert(0, "/opt/trn_rl_repo")

import numpy as np
import ml_dtypes

import concourse.bass as bass  # noqa: F401
import concourse.mybir as mybir
import concourse.tile as tile
from concourse import bacc, bass2jax

# Problem dims (hardcoded per contract)
IN, HID, OUT, PART, E = 1024, 4096, 1024, 256, 8
B, N_TOK = 16, 2048
NCORES = 8
BPC = B // NCORES            # batch elements per core
T_CORE = BPC * N_TOK         # tokens per core
TT = 2048                    # token tile = one batch element
KC = IN // 128               # fc1 contraction chunks
MC = HID // 128              # hidden chunks
CC = OUT // 128              # output col chunks
SS = TT // 512               # 512-token subtiles per tile
F32 = mybir.dt.float32
BF16 = mybir.dt.bfloat16
NP_BF16 = ml_dtypes.bfloat16

_CACHE = {}


def _strip_redundant_ldweights(nc):
    """Remove InstLdweights whose stationary AP matches the immediately
    preceding Ldweights on the PE stream with only plain matmuls in
    between: the PE array already holds those weights. Dependencies of a
    stripped load are merged into the following matmul."""
    def ldkey(inst):
        a = inst.ins[0]
        return (
            str(a.memref), a.offset, str(a.ap), str(a.dtype),
            str(inst.perf_mode), str(inst.is_transpose),
        )

    n_stripped = 0
    for blk in nc.m.functions[0].blocks:
        il = blk.instructions  # live list
        prev_key = None
        only_mm_since = True
        i = 0
        while i < len(il):
            inst = il[i]
            if getattr(inst, "engine", None) != mybir.EngineType.PE:
                i += 1
                continue
            if isinstance(inst, mybir.InstLdweights):
                k = ldkey(inst)
                nxt = il[i + 1] if i + 1 < len(il) else None
                if (
                    k == prev_key
                    and only_mm_since
                    and not inst.has_wait()
                    and not inst.has_update()
                    and isinstance(nxt, mybir.InstMatmult)
                ):
                    try:
                        nxt.merge_dependencies_from(inst)
                    except Exception:
                        nxt.add_sync_dependencies_from(inst)
                        nxt.add_nosync_dependencies_from(inst)
                    del il[i]
                    n_stripped += 1
                    continue
                prev_key = k
                only_mm_since = True
            elif isinstance(inst, mybir.InstMatmult):
                pass
            else:
                only_mm_since = False
            i += 1
    return n_stripped


def _build_nc(reps=1, hw_loop=False, hw_loop_kwargs=None, unroll=1):
    nc = bacc.Bacc(None, target_bir_lowering=False, debug=False)

    xt_d = nc.dram_tensor("xt", [IN, T_CORE], BF16, kind="ExternalInput")
    w1_d = nc.dram_tensor("w1", [MC, 128, IN], BF16, kind="ExternalInput")
    b1_d = nc.dram_tensor("b1r", [128, MC], F32, kind="ExternalInput")
    wc_d = nc.dram_tensor("wc", [BPC, CC, 128, HID], BF16, kind="ExternalInput")
    bb_d = nc.dram_tensor("bb", [BPC, 128, CC], F32, kind="ExternalInput")
    out_d = nc.dram_tensor("out", [BPC, OUT, N_TOK], BF16, kind="ExternalOutput")

    with tile.TileContext(nc) as tc:
        with (
            tc.tile_pool(name="const", bufs=1) as cpool,
            tc.tile_pool(name="h", bufs=MC) as hpool,
            tc.tile_pool(name="x", bufs=KC) as xpool,
            tc.tile_pool(name="w1", bufs=4) as w1pool,
            tc.tile_pool(name="wc", bufs=2) as wcpool,
            tc.tile_pool(name="o", bufs=2) as opool,
            tc.tile_pool(name="ps", bufs=8, space="PSUM") as pspool,
        ):
            b1_sb = cpool.tile([128, MC], F32, tag="b1")
            nc.sync.dma_start(b1_sb[:], b1_d[:])
            bb_sb = []
            for j in range(BPC):
                t_ = cpool.tile([128, CC], F32, tag=f"bb{j}")
                nc.sync.dma_start(t_[:], bb_d[j])
                bb_sb.append(t_)

            def one_rep():
                for j in range(BPC):
                    # ---- Phase A: hT[hid, tok] = gelu(W1.T @ xT + b1) ----
                    xts = []
                    for k in range(KC):
                        xk = xpool.tile([128, TT], BF16, tag="x")
                        nc.sync.dma_start(
                            xk[:],
                            xt_d[k * 128 : (k + 1) * 128, j * TT : (j + 1) * TT],
                        )
                        xts.append(xk)

                    hts = []
                    for m in range(MC):
                        w1m = w1pool.tile([128, IN], BF16, tag="w1")
                        nc.sync.dma_start(w1m[:], w1_d[m])
                        hm = hpool.tile([128, TT], BF16, tag="h")
                        psa = [
                            pspool.tile([128, 512], F32, tag="ps", name=f"psa{s}")
                            for s in range(SS)
                        ]
                        # k-outer / s-inner: each stationary W1 (m,k) chunk
                        # feeds 4 token subtiles back-to-back
                        for k in range(KC):
                            for s in range(SS):
                                nc.tensor.matmul(
                                    psa[s][:],
                                    w1m[:, k * 128 : (k + 1) * 128],
                                    xts[k][:, s * 512 : (s + 1) * 512],
                                    start=(k == 0),
                                    stop=(k == KC - 1),
                                )
                        for s in range(SS):
                            nc.scalar.activation(
                                hm[:, s * 512 : (s + 1) * 512],
                                psa[s][:],
                                mybir.ActivationFunctionType.Gelu,
                                bias=b1_sb[:, m : m + 1],
                            )
                        hts.append(hm)

                    # ---- Phase B: outT[col, tok] = Wcat.T @ h + bcat ----
                    for c in range(CC):
                        wc = wcpool.tile([128, HID], BF16, tag="wc")
                        nc.sync.dma_start(wc[:], wc_d[j, c])
                        pso = [
                            pspool.tile([128, 512], F32, tag="ps", name=f"pso{u}")
                            for u in range(SS)
                        ]
                        # m-outer / u-inner: each stationary Wcat (c,m) chunk
                        # feeds 4 token subtiles back-to-back
                        for m in range(MC):
                            for u in range(SS):
                                nc.tensor.matmul(
                                    pso[u][:],
                                    wc[:, m * 128 : (m + 1) * 128],
                                    hts[m][:, u * 512 : (u + 1) * 512],
                                    start=(m == 0),
                                    stop=(m == MC - 1),
                                )
                        ob = opool.tile([128, TT], BF16, tag="o")
                        for u in range(SS):
                            nc.scalar.activation(
                                ob[:, u * 512 : (u + 1) * 512],
                                pso[u][:],
                                mybir.ActivationFunctionType.Identity,
                                bias=bb_sb[j][:, c : c + 1],
                            )
                        nc.sync.dma_start(
                            out_d[j, c * 128 : (c + 1) * 128, :], ob[:]
                        )

            if hw_loop:
                assert reps % unroll == 0
                with tc.For_i(0, reps // unroll, **(hw_loop_kwargs or {})):
                    for _ in range(unroll):
                        one_rep()
            else:
                for _ in range(reps):
                    one_rep()

    n = _strip_redundant_ldweights(nc)
    print(f"v3: stripped {n} redundant ldweights")
    nc.compile()
    return nc


def _make_runner(nc):
    """Cached executor mirroring bass2jax.run_bass_via_pjrt's multi-core
    path, but reusable; uses the C++ fast dispatch path when available."""
    import jax
    from jax.experimental.shard_map import shard_map
    from jax.sharding import Mesh, NamedSharding, PartitionSpec

    bass2jax.install_neuronx_cc_hook()

    partition_name = (
        nc.partition_id_tensor.name if nc.partition_id_tensor else None
    )
    in_names, out_names, out_avals, zero_outs = [], [], [], []
    for alloc in nc.m.functions[0].allocations:
        if not isinstance(alloc, mybir.MemoryLocationSet):
            continue
        name = alloc.memorylocations[0].name
        if alloc.kind == "ExternalInput":
            if name != partition_name:
                in_names.append(name)
        elif alloc.kind == "ExternalOutput":
            out_avals.append(
                jax.core.ShapedArray(alloc.tensor_shape, mybir.dt.np(alloc.dtype))
            )
            zero_outs.append(
                np.zeros(alloc.tensor_shape, dtype=mybir.dt.np(alloc.dtype))
            )
            out_names.append(name)

    n_params = len(in_names)
    all_names = in_names + out_names
    if partition_name is not None:
        all_names = all_names + [partition_name]

    def _body(*args):
        operands = list(args)
        if partition_name is not None:
            operands.append(bass2jax.partition_id_tensor())
        outs = bass2jax._bass_exec_p.bind(
            *operands,
            out_avals=tuple(out_avals),
            in_names=tuple(all_names),
            out_names=tuple(out_names),
            lowering_input_output_aliases=(),
            sim_require_finite=True,
            sim_require_nnan=True,
            nc=nc,
        )
        return tuple(outs)

    devices = jax.devices()[:NCORES]
    mesh = Mesh(np.asarray(devices), ("core",))
    spec = NamedSharding(mesh, PartitionSpec("core"))
    n_outs = len(out_names)

    def make_jit():
        return jax.jit(
            shard_map(
                _body,
                mesh=mesh,
                in_specs=(PartitionSpec("core"),) * (n_params + n_outs),
                out_specs=(PartitionSpec("core"),) * n_outs,
                check_rep=False,
            ),
            donate_argnums=tuple(range(n_params, n_params + n_outs)),
            keep_unused=True,
        )

    def _global_struct(per_core_shape, dtype):
        return jax.ShapeDtypeStruct(
            (NCORES * per_core_shape[0], *per_core_shape[1:]), dtype, sharding=spec
        )

    def put_inputs(in_maps):
        concat = [
            np.concatenate([np.asarray(m[name]) for m in in_maps], axis=0)
            for name in in_names
        ]
        return [jax.device_put(a, spec) for a in concat]

    def put_zeros():
        return [
            jax.device_put(
                np.zeros((NCORES * z.shape[0], *z.shape[1:]), z.dtype), spec
            )
            for z in zero_outs
        ]

    sharded = None
    try:
        arg_structs = []
        for name in in_names:
            for alloc in nc.m.functions[0].allocations:
                if (
                    isinstance(alloc, mybir.MemoryLocationSet)
                    and alloc.memorylocations[0].name == name
                ):
                    arg_structs.append(
                        _global_struct(alloc.tensor_shape, mybir.dt.np(alloc.dtype))
                    )
                    break
        for z in zero_outs:
            arg_structs.append(_global_struct(z.shape, z.dtype))
        sharded = bass2jax.fast_dispatch_compile(
            lambda: make_jit().lower(*arg_structs).compile()
        )
    except Exception as e:  # pragma: no cover
        print(f"fast_dispatch unavailable ({type(e).__name__}: {e}); using jit")
        sharded = make_jit()

    def run(dev_inputs, dev_zeros):
        out_arrs = sharded(*dev_inputs, *dev_zeros)
        return [
            {
                name: np.asarray(out_arrs[i]).reshape(
                    NCORES, *out_avals[i].shape
                )[c]
                for i, name in enumerate(out_names)
            }
            for c in range(NCORES)
        ]

    return {
        "run": run,
        "put_inputs": put_inputs,
        "put_zeros": put_zeros,
        "sharded": sharded,
        "out_names": out_names,
    }


def get_runner():
    if "nc" not in _CACHE:
        _CACHE["nc"] = _build_nc()
    if "runner" not in _CACHE:
        _CACHE["runner"] = _make_runner(_CACHE["nc"])
    return _CACHE["runner"]


def make_in_maps(x, indices, W1, b1, W2, b2, We, be):
    # w1r[m, p, k*128+q] = W1[k*128+p, m*128+q]: stationary chunk (m,k)
    # = w1r[m][:, k*128:(k+1)*128] with partitions = IN-chunk-k rows.
    w1r = np.ascontiguousarray(
        W1.reshape(KC, 128, MC, 128).transpose(2, 1, 0, 3).reshape(MC, 128, IN)
    ).astype(NP_BF16)
    b1r = np.ascontiguousarray(b1.reshape(MC, 128).T)

    in_maps = []
    for c in range(NCORES):
        xt = np.ascontiguousarray(
            x[c * BPC : (c + 1) * BPC].reshape(T_CORE, IN).T
        ).astype(NP_BF16)
        wc = np.empty((BPC, CC, 128, HID), dtype=NP_BF16)
        bb = np.empty((BPC, 128, CC), dtype=np.float32)
        for jj in range(BPC):
            e = int(indices[c * BPC + jj])
            wcat = np.concatenate([W2, We[e]], axis=1)  # [HID, OUT]
            # wc[jj, cc, p, m*128+q] = wcat[m*128+p, cc*128+q]
            wc[jj] = (
                wcat.reshape(MC, 128, CC, 128)
                .transpose(2, 1, 0, 3)
                .reshape(CC, 128, HID)
                .astype(NP_BF16)
            )
            bb[jj] = np.concatenate([b2, be[e]]).reshape(CC, 128).T
        in_maps.append({"xt": xt, "w1": w1r, "b1r": b1r, "wc": wc, "bb": bb})
    return in_maps


def kernel(x, indices, W1, b1, W2, b2, We, be):
    x = np.ascontiguousarray(np.asarray(x, dtype=np.float32))
    indices = np.asarray(indices).astype(np.int64)
    W1 = np.asarray(W1, dtype=np.float32)
    b1 = np.asarray(b1, dtype=np.float32)
    W2 = np.asarray(W2, dtype=np.float32)
    b2 = np.asarray(b2, dtype=np.float32)
    We = np.asarray(We, dtype=np.float32)
    be = np.asarray(be, dtype=np.float32)

    runner = get_runner()
    in_maps = make_in_maps(x, indices, W1, b1, W2, b2, We, be)
    dev_in = runner["put_inputs"](in_maps)
    results = runner["run"](dev_in, runner["put_zeros"]())

    out = np.empty((B, N_TOK, OUT), dtype=np.float32)
    for c in range(NCORES):
        # device output is [BPC, OUT, N_TOK] bf16, column-major
        ot = results[c]["out"].astype(np.float32)
        out[c * BPC : (c + 1) * BPC] = ot.transpose(0, 2, 1)
    return out


# revision 5
# speedup vs baseline: 1.2128x; 1.0208x over previous
"""MoE MLP kernel for Trainium2 (8 NeuronCores, Bass/Tile) — v3.

Problem: y = concat(h @ W2 + b2, h @ We[idx_b] + be[idx_b]) where
h = gelu(x @ W1 + b1), x: [16, 2048, 1024] f32, W1: [1024, 4096],
W2: [4096, 768], We: [8, 4096, 256], idx: [16] in [0, 8).

Sharding: data-parallel over batch B=16 -> 2 batch elements per core.
Expert selection resolved on host: each core gets [W2 | We_sel] packed
per batch element -> one dense GEMM pipeline, no collectives.

v3 changes vs v2:
  * token tile TT=2048 = one full batch element per tile, so every
    stationary weight chunk feeds FOUR N=512 matmuls (was two) in both
    phases -> half the LDWEIGHTS instructions.
  * redundant LDWEIGHTS (same stationary AP, only matmuls in between)
    are stripped post-schedule (4096 -> 1024 remain). Measured effect on
    HW time is ~nil (the PE hides weight reloads behind matmul drain),
    but the strip shrinks the NEFF ~20% and costs nothing.
  * phase B swaps stationary/moving: stationary = Wcat column-chunk
    [128 hid, 128 col] (prepacked per (batch, col-chunk) as one
    contiguous [128, 4096] slab = one 1 MB DMA), moving = h token
    slices. Output lands column-major [OUT, N_TOK]; host untransposes.
  * outputs stored bf16 (halves output DMA; rounding adds ~0.2% per
    element, far inside the 2e-2 gate).
  * per-phase DMA per core drops to x 8.4 + W1 16.8 + Wcat 16.8 +
    out 8.4 = 50 MB/rep, fully hidden under ~0.9 ms of PE work.

Per-core pipeline per batch element (2048 tokens):
  Phase A: for m (32 hid chunks): psum[s], s=0..3 accumulate over k=8
    IN-chunks; each stationary W1 (m,k) chunk feeds 4 token subtiles
    back-to-back. ScalarE applies bias+gelu on the PSUM->SBUF move,
    writing h[m] [128, 2048] bf16 (128 KiB/partition total, resident).
  Phase B: for c (8 col chunks): psum[u], u=0..3 accumulate over m=32
    hid chunks; each stationary Wcat (c,m) chunk feeds 4 token subtiles.
    ScalarE fuses bias-add with the PSUM->SBUF copy (bf16); one 512 KB
    DMA per (batch, col chunk) stores [128, 2048] to DRAM.
"""

import sys

sys.path.insert(0, "/opt/trn_rl_repo")

import numpy as np
import ml_dtypes

import concourse.bass as bass  # noqa: F401
import concourse.mybir as mybir
import concourse.tile as tile
from concourse import bacc, bass2jax

# Problem dims (hardcoded per contract)
IN, HID, OUT, PART, E = 1024, 4096, 1024, 256, 8
B, N_TOK = 16, 2048
NCORES = 8
BPC = B // NCORES            # batch elements per core
T_CORE = BPC * N_TOK         # tokens per core
TT = 2048                    # token tile = one batch element
KC = IN // 128               # fc1 contraction chunks
MC = HID // 128              # hidden chunks
CC = OUT // 128              # output col chunks
SS = TT // 512               # 512-token subtiles per tile
F32 = mybir.dt.float32
BF16 = mybir.dt.bfloat16
NP_BF16 = ml_dtypes.bfloat16

_CACHE = {}


def _strip_redundant_ldweights(nc):
    """Remove InstLdweights whose stationary AP matches the immediately
    preceding Ldweights on the PE stream with only plain matmuls in
    between: the PE array already holds those weights. Dependencies of a
    stripped load are merged into the following matmul."""
    def ldkey(inst):
        a = inst.ins[0]
        return (
            str(a.memref), a.offset, str(a.ap), str(a.dtype),
            str(inst.perf_mode), str(inst.is_transpose),
        )

    n_stripped = 0
    for blk in nc.m.functions[0].blocks:
        il = blk.instructions  # live list
        prev_key = None
        only_mm_since = True
        i = 0
        while i < len(il):
            inst = il[i]
            if getattr(inst, "engine", None) != mybir.EngineType.PE:
                i += 1
                continue
            if isinstance(inst, mybir.InstLdweights):
                k = ldkey(inst)
                nxt = il[i + 1] if i + 1 < len(il) else None
                if (
                    k == prev_key
                    and only_mm_since
                    and not inst.has_wait()
                    and not inst.has_update()
                    and isinstance(nxt, mybir.InstMatmult)
                ):
                    try:
                        nxt.merge_dependencies_from(inst)
                    except Exception:
                        nxt.add_sync_dependencies_from(inst)
                        nxt.add_nosync_dependencies_from(inst)
                    del il[i]
                    n_stripped += 1
                    continue
                prev_key = k
                only_mm_since = True
            elif isinstance(inst, mybir.InstMatmult):
                pass
            else:
                only_mm_since = False
            i += 1
    return n_stripped


def _build_nc(reps=1, hw_loop=False, hw_loop_kwargs=None, unroll=1):
    nc = bacc.Bacc(None, target_bir_lowering=False, debug=False)

    xt_d = nc.dram_tensor("xt", [IN, T_CORE], BF16, kind="ExternalInput")
    w1_d = nc.dram_tensor("w1", [MC, 128, IN], BF16, kind="ExternalInput")
    b1_d = nc.dram_tensor("b1r", [128, MC], F32, kind="ExternalInput")
    wc_d = nc.dram_tensor("wc", [BPC, CC, 128, HID], BF16, kind="ExternalInput")
    bb_d = nc.dram_tensor("bb", [BPC, 128, CC], F32, kind="ExternalInput")
    out_d = nc.dram_tensor("out", [BPC, OUT, N_TOK], BF16, kind="ExternalOutput")

    with tile.TileContext(nc) as tc:
        with (
            tc.tile_pool(name="const", bufs=1) as cpool,
            tc.tile_pool(name="h", bufs=MC) as hpool,
            tc.tile_pool(name="x", bufs=KC) as xpool,
            tc.tile_pool(name="w1", bufs=4) as w1pool,
            tc.tile_pool(name="wc", bufs=2) as wcpool,
            tc.tile_pool(name="o", bufs=2) as opool,
            tc.tile_pool(name="ps", bufs=8, space="PSUM") as pspool,
        ):
            b1_sb = cpool.tile([128, MC], F32, tag="b1")
            nc.sync.dma_start(b1_sb[:], b1_d[:])
            bb_sb = []
            for j in range(BPC):
                t_ = cpool.tile([128, CC], F32, tag=f"bb{j}")
                nc.sync.dma_start(t_[:], bb_d[j])
                bb_sb.append(t_)

            def one_rep():
                for j in range(BPC):
                    # ---- Phase A: hT[hid, tok] = gelu(W1.T @ xT + b1) ----
                    xts = []
                    for k in range(KC):
                        xk = xpool.tile([128, TT], BF16, tag="x")
                        nc.sync.dma_start(
                            xk[:],
                            xt_d[k * 128 : (k + 1) * 128, j * TT : (j + 1) * TT],
                        )
                        xts.append(xk)

                    hts = []
                    for m in range(MC):
                        w1m = w1pool.tile([128, IN], BF16, tag="w1")
                        nc.sync.dma_start(w1m[:], w1_d[m])
                        hm = hpool.tile([128, TT], BF16, tag="h")
                        psa = [
                            pspool.tile([128, 512], F32, tag="ps", name=f"psa{s}")
                            for s in range(SS)
                        ]
                        # k-outer / s-inner: each stationary W1 (m,k) chunk
                        # feeds 4 token subtiles back-to-back
                        for k in range(KC):
                            for s in range(SS):
                                nc.tensor.matmul(
                                    psa[s][:],
                                    w1m[:, k * 128 : (k + 1) * 128],
                                    xts[k][:, s * 512 : (s + 1) * 512],
                                    start=(k == 0),
                                    stop=(k == KC - 1),
                                )
                        for s in range(SS):
                            nc.scalar.activation(
                                hm[:, s * 512 : (s + 1) * 512],
                                psa[s][:],
                                mybir.ActivationFunctionType.Gelu,
                                bias=b1_sb[:, m : m + 1],
                            )
                        hts.append(hm)

                    # ---- Phase B: outT[col, tok] = Wcat.T @ h + bcat ----
                    for c in range(CC):
                        wc = wcpool.tile([128, HID], BF16, tag="wc")
                        nc.sync.dma_start(wc[:], wc_d[j, c])
                        pso = [
                            pspool.tile([128, 512], F32, tag="ps", name=f"pso{u}")
                            for u in range(SS)
                        ]
                        # m-outer / u-inner: each stationary Wcat (c,m) chunk
                        # feeds 4 token subtiles back-to-back
                        for m in range(MC):
                            for u in range(SS):
                                nc.tensor.matmul(
                                    pso[u][:],
                                    wc[:, m * 128 : (m + 1) * 128],
                                    hts[m][:, u * 512 : (u + 1) * 512],
                                    start=(m == 0),
                                    stop=(m == MC - 1),
                                )
                        ob = opool.tile([128, TT], BF16, tag="o")
                        for u in range(SS):
                            nc.scalar.activation(
                                ob[:, u * 512 : (u + 1) * 512],
                                pso[u][:],
                                mybir.ActivationFunctionType.Identity,
                                bias=bb_sb[j][:, c : c + 1],
                            )
                        nc.sync.dma_start(
                            out_d[j, c * 128 : (c + 1) * 128, :], ob[:]
                        )

            if hw_loop:
                assert reps % unroll == 0
                with tc.For_i(0, reps // unroll, **(hw_loop_kwargs or {})):
                    for _ in range(unroll):
                        one_rep()
            else:
                for _ in range(reps):
                    one_rep()

    n = _strip_redundant_ldweights(nc)
    print(f"v3: stripped {n} redundant ldweights")
    nc.compile()
    return nc


def _make_runner(nc):
    """Cached executor mirroring bass2jax.run_bass_via_pjrt's multi-core
    path, but reusable; uses the C++ fast dispatch path when available."""
    import jax
    from jax.experimental.shard_map import shard_map
    from jax.sharding import Mesh, NamedSharding, PartitionSpec

    bass2jax.install_neuronx_cc_hook()

    partition_name = (
        nc.partition_id_tensor.name if nc.partition_id_tensor else None
    )
    in_names, out_names, out_avals, zero_outs = [], [], [], []
    for alloc in nc.m.functions[0].allocations:
        if not isinstance(alloc, mybir.MemoryLocationSet):
            continue
        name = alloc.memorylocations[0].name
        if alloc.kind == "ExternalInput":
            if name != partition_name:
                in_names.append(name)
        elif alloc.kind == "ExternalOutput":
            out_avals.append(
                jax.core.ShapedArray(alloc.tensor_shape, mybir.dt.np(alloc.dtype))
            )
            zero_outs.append(
                np.zeros(alloc.tensor_shape, dtype=mybir.dt.np(alloc.dtype))
            )
            out_names.append(name)

    n_params = len(in_names)
    all_names = in_names + out_names
    if partition_name is not None:
        all_names = all_names + [partition_name]

    def _body(*args):
        operands = list(args)
        if partition_name is not None:
            operands.append(bass2jax.partition_id_tensor())
        outs = bass2jax._bass_exec_p.bind(
            *operands,
            out_avals=tuple(out_avals),
            in_names=tuple(all_names),
            out_names=tuple(out_names),
            lowering_input_output_aliases=(),
            sim_require_finite=True,
            sim_require_nnan=True,
            nc=nc,
        )
        return tuple(outs)

    devices = jax.devices()[:NCORES]
    mesh = Mesh(np.asarray(devices), ("core",))
    spec = NamedSharding(mesh, PartitionSpec("core"))
    n_outs = len(out_names)

    def make_jit():
        return jax.jit(
            shard_map(
                _body,
                mesh=mesh,
                in_specs=(PartitionSpec("core"),) * (n_params + n_outs),
                out_specs=(PartitionSpec("core"),) * n_outs,
                check_rep=False,
            ),
            donate_argnums=tuple(range(n_params, n_params + n_outs)),
            keep_unused=True,
        )

    def _global_struct(per_core_shape, dtype):
        return jax.ShapeDtypeStruct(
            (NCORES * per_core_shape[0], *per_core_shape[1:]), dtype, sharding=spec
        )

    def put_inputs(in_maps):
        concat = [
            np.concatenate([np.asarray(m[name]) for m in in_maps], axis=0)
            for name in in_names
        ]
        return [jax.device_put(a, spec) for a in concat]

    def put_zeros():
        return [
            jax.device_put(
                np.zeros((NCORES * z.shape[0], *z.shape[1:]), z.dtype), spec
            )
            for z in zero_outs
        ]

    sharded = None
    try:
        arg_structs = []
        for name in in_names:
            for alloc in nc.m.functions[0].allocations:
                if (
                    isinstance(alloc, mybir.MemoryLocationSet)
                    and alloc.memorylocations[0].name == name
                ):
                    arg_structs.append(
                        _global_struct(alloc.tensor_shape, mybir.dt.np(alloc.dtype))
                    )
                    break
        for z in zero_outs:
            arg_structs.append(_global_struct(z.shape, z.dtype))
        sharded = bass2jax.fast_dispatch_compile(
            lambda: make_jit().lower(*arg_structs).compile()
        )
    except Exception as e:  # pragma: no cover
        print(f"fast_dispatch unavailable ({type(e).__name__}: {e}); using jit")
        sharded = make_jit()

    def run(dev_inputs, dev_zeros):
        out_arrs = sharded(*dev_inputs, *dev_zeros)
        return [
            {
                name: np.asarray(out_arrs[i]).reshape(
                    NCORES, *out_avals[i].shape
                )[c]
                for i, name in enumerate(out_names)
            }
            for c in range(NCORES)
        ]

    return {
        "run": run,
        "put_inputs": put_inputs,
        "put_zeros": put_zeros,
        "sharded": sharded,
        "out_names": out_names,
    }


def get_runner():
    if "nc" not in _CACHE:
        _CACHE["nc"] = _build_nc()
    if "runner" not in _CACHE:
        _CACHE["runner"] = _make_runner(_CACHE["nc"])
    return _CACHE["runner"]


def make_in_maps(x, indices, W1, b1, W2, b2, We, be):
    # w1r[m, p, k*128+q] = W1[k*128+p, m*128+q]: stationary chunk (m,k)
    # = w1r[m][:, k*128:(k+1)*128] with partitions = IN-chunk-k rows.
    w1r = np.ascontiguousarray(
        W1.reshape(KC, 128, MC, 128).transpose(2, 1, 0, 3).reshape(MC, 128, IN)
    ).astype(NP_BF16)
    b1r = np.ascontiguousarray(b1.reshape(MC, 128).T)

    in_maps = []
    for c in range(NCORES):
        xt = np.ascontiguousarray(
            x[c * BPC : (c + 1) * BPC].reshape(T_CORE, IN).T
        ).astype(NP_BF16)
        wc = np.empty((BPC, CC, 128, HID), dtype=NP_BF16)
        bb = np.empty((BPC, 128, CC), dtype=np.float32)
        for jj in range(BPC):
            e = int(indices[c * BPC + jj])
            wcat = np.concatenate([W2, We[e]], axis=1)  # [HID, OUT]
            # wc[jj, cc, p, m*128+q] = wcat[m*128+p, cc*128+q]
            wc[jj] = (
                wcat.reshape(MC, 128, CC, 128)
                .transpose(2, 1, 0, 3)
                .reshape(CC, 128, HID)
                .astype(NP_BF16)
            )
            bb[jj] = np.concatenate([b2, be[e]]).reshape(CC, 128).T
        in_maps.append({"xt": xt, "w1": w1r, "b1r": b1r, "wc": wc, "bb": bb})
    return in_maps


def kernel(x, indices, W1, b1, W2, b2, We, be):
    x = np.ascontiguousarray(np.asarray(x, dtype=np.float32))
    indices = np.asarray(indices).astype(np.int64)
    W1 = np.asarray(W1, dtype=np.float32)
    b1 = np.asarray(b1, dtype=np.float32)
    W2 = np.asarray(W2, dtype=np.float32)
    b2 = np.asarray(b2, dtype=np.float32)
    We = np.asarray(We, dtype=np.float32)
    be = np.asarray(be, dtype=np.float32)

    runner = get_runner()
    in_maps = make_in_maps(x, indices, W1, b1, W2, b2, We, be)
    dev_in = runner["put_inputs"](in_maps)
    results = runner["run"](dev_in, runner["put_zeros"]())

    out = np.empty((B, N_TOK, OUT), dtype=np.float32)
    for c in range(NCORES):
        # device output is [BPC, OUT, N_TOK] bf16, column-major
        ot = results[c]["out"].astype(np.float32)
        out[c * BPC : (c + 1) * BPC] = ot.transpose(0, 2, 1)
    return out
